# revision 22
# baseline (speedup 1.0000x reference)
"""GCN 2-layer kernel for Trainium2 (8 NeuronCores, Bass/Tile), v2.

Strategy (vs v1 which used dma_scatter_add into DRAM accumulators):
  - Nodes sharded across 8 cores (degree-balanced snake), S=13312 slots/core
    = 104 blocks of 128 nodes.
  - Per layer: a node table g (g1 = dis*(x@W1), g2 = dis*z) is computed
    per-shard, AllGathered to a full [106496, 64] fp32 DRAM table.
    NOTE: the layer-2 W2 matmul is moved AFTER aggregation (linearity),
    so the layer-2 table is just dis*z.
  - Edge phase: edges (plus self-loops) are grouped by destination block
    (128 dsts) and source range (16K rows, int16 gather index limit), with
    token counts padded to a cross-core-uniform static schedule.
    dma_gather pulls source rows (256B each) into SBUF tiles of
    [128 tokens, 64 feats]; the TensorEngine segment-reduces each tile into
    a feature-major PSUM accumulator [64, 1536] (24-block superblock pairs
    on partition halves) via tile-built 0/1 matrices
    M[token, dst_off] = (dstoff[token] == iota), streamed as matmul rhs.
    PSUM banks are cleared by outer-product zero-matmuls (start=True),
    real matmuls accumulate with start=False.
  - Tails per 128-dst block: transpose to node-major via PE, then
    layer 1: z~ = dis*relu(dis*agg + b1) -> layer-2 table;
    layer 2: h2 = (agg @ W2), y = log_softmax(dis*h2 + b2).
"""
import os
import sys

sys.path.insert(0, "/opt/trn_rl_repo")

import numpy as np

N, FIN, HD, C = 100000, 128, 64, 64
E = 1600000
NCORES = 8
S = 13312                  # slots per core
NT = S // 128              # 104 dst blocks
GROWS = NCORES * S         # 106496 global table rows
RANGE_W = int(os.environ.get("KRW", "16384"))  # gather index range (int16)
NR = (GROWS + RANGE_W - 1) // RANGE_W
PAIRW = 24                 # dst blocks per superblock-pair (2 halves x 12)
NSBP = (NT + PAIRW - 1) // PAIRW          # 5 (24,24,24,24,8)
HALF_BLKS = PAIRW // 2     # 12
HALF_COLS = HALF_BLKS * 128  # 1536 psum cols per half (3 banks)
BATCH = 1024               # max tokens per gather
NQ = 4


def _plan(x, edge_index):
    """Host-side planning. Returns per-core arrays + a uniform schedule."""
    x = np.asarray(x, np.float32)
    ei = np.asarray(edge_index, np.int64)
    src, dst = ei[0], ei[1]

    deg = np.bincount(dst, minlength=N).astype(np.float64) + 1.0  # + self loop
    dis_f = (1.0 / np.sqrt(deg)).astype(np.float32)

    # degree-balanced snake sharding
    order = np.argsort(-deg, kind="stable")
    posn = np.arange(N)
    blk, rem = posn // NCORES, posn % NCORES
    corepat = np.where(blk % 2 == 0, rem, NCORES - 1 - rem).astype(np.int32)
    core_of = np.empty(N, np.int32)
    core_of[order] = corepat

    lrow_of = np.empty(N, np.int64)
    nodes_by_core = []
    for c in range(NCORES):
        nodes_c = order[core_of[order] == c]
        lrow_of[nodes_c] = np.arange(len(nodes_c))
        nodes_by_core.append(nodes_c)

    grow_of = core_of.astype(np.int64) * S + lrow_of

    # per-core token lists: real edges only (self loops get dedicated
    # "self" groups gathering from the local bounce tensor)
    tok_src = []   # global row of source
    tok_dst = []   # local row of dest
    dst_core = core_of[dst]
    for c in range(NCORES):
        m = dst_core == c
        tok_src.append(grow_of[src[m]])
        tok_dst.append(lrow_of[dst[m]])

    # counts per (core, block, range)
    cnt = np.zeros((NCORES, NT, NR), np.int64)
    for c in range(NCORES):
        b = tok_dst[c] >> 7
        r = tok_src[c] // RANGE_W
        np.add.at(cnt[c], (b, r), 1)

    # uniform padded tile counts per (block, range)
    ntok = cnt.max(axis=0)                       # [NT, NR]
    ntiles_br = -(-ntok // 128)                  # ceil to 128-token tiles

    # schedule: per sbp, per-range edge groups (self-loop contributions are
    # added in the tails from direct contiguous loads of the bounce tensor).
    # Tiles in block-major order; gathers chunk them by 8 (last partial).
    sched = []            # per (sbp, r): dict(tiles=[(b)], ngather)
    total_tiles = 0
    for sbp in range(NSBP):
        b0, b1 = sbp * PAIRW, min((sbp + 1) * PAIRW, NT)
        for r in range(NR):
            tiles = []
            for b in range(b0, b1):
                tiles += [b] * int(ntiles_br[b, r])
            ng = -(-len(tiles) // 8) if tiles else 0
            sched.append({
                "sbp": sbp, "r": r, "tiles": tiles, "ngather": ng,
            })
            total_tiles += len(tiles)

    ngather_total = sum(g["ngather"] for g in sched)
    nmm_total = total_tiles

    # build per-core gidx + dstoff arrays following the schedule
    def wrap16(v):
        n = len(v)
        a = np.asarray(v, np.int16).reshape(n // 16, 16).T.copy()
        return np.tile(a, (8, 1))

    gidx_all = np.zeros((NCORES, 128, ngather_total * (BATCH // 16)), np.int16)
    doff_all = np.full((NCORES, 128, max(nmm_total, 1)), -1.0, np.float32)

    for c in range(NCORES):
        b_of = tok_dst[c] >> 7
        r_of = tok_src[c] // RANGE_W
        # bucket tokens by (block, range)
        key = b_of * NR + r_of
        osort = np.argsort(key, kind="stable")
        ts_s, td_s = tok_src[c][osort], tok_dst[c][osort]
        key_s = key[osort]
        starts = np.searchsorted(key_s, np.arange(NT * NR))
        ends = np.searchsorted(key_s, np.arange(NT * NR) + 1)

        g_cursor = 0   # in gathers
        m_cursor = 0   # in mms (= tiles)
        for grp in sched:
            r = grp["r"]
            tiles = grp["tiles"]
            if not tiles:
                continue
            # assemble this group's token stream (tile-major, 128 per tile)
            n_tok = len(tiles) * 128
            idx_loc = np.zeros(n_tok, np.int64)   # gather idx within range
            dof = np.full(n_tok, -1.0, np.float32)
            # fill per block
            tpos = 0
            tiles_arr = np.asarray(tiles)
            for b in np.unique(tiles_arr):
                k = b * NR + r
                s0, s1 = starts[k], ends[k]
                cnt_b = s1 - s0
                nt_b = int(np.count_nonzero(tiles_arr == b))
                # tokens for this block go into its nt_b tiles
                off = tpos * 128
                idx_loc[off:off + cnt_b] = ts_s[s0:s1] - r * RANGE_W
                dof[off:off + cnt_b] = (
                    td_s[s0:s1] - b * 128).astype(np.float32)
                tpos += nt_b
            # scatter into gidx (per gather chunk) and dstoff (per tile)
            n_tiles = len(tiles)
            for g in range(grp["ngather"]):
                t0, t1 = g * 8, min((g + 1) * 8, n_tiles)
                nidx = (t1 - t0) * 128
                seg = idx_loc[t0 * 128: t0 * 128 + nidx]
                col0 = (g_cursor + g) * (BATCH // 16)
                w = wrap16(np.pad(seg, (0, BATCH - nidx)))
                gidx_all[c][:, col0:col0 + BATCH // 16] = w
            doff_all[c][:, m_cursor:m_cursor + n_tiles] = (
                dof.reshape(n_tiles, 128).T)
            g_cursor += grp["ngather"]
            m_cursor += n_tiles

    # per-core xT, dis with p = row%128, t = row//128
    xT_all = np.zeros((NCORES, 128, S), np.float32)
    dis_all = np.ones((NCORES, 128, NT), np.float32)
    for c in range(NCORES):
        nodes_c = nodes_by_core[c]
        n_real = len(nodes_c)
        lin = np.arange(n_real)
        xT_all[c][:, lin] = x[nodes_c].T
        dis_all[c][lin % 128, lin // 128] = dis_f[nodes_c]

    # consts tile: [128, 640]: cols 0:128 iota, 128:640 zeros
    consts = np.zeros((128, 640), np.float32)
    consts[:, 0:128] = np.arange(128, dtype=np.float32)[None, :]

    return {
        "sched": sched,
        "ngather_total": ngather_total,
        "nmm_total": nmm_total,
        "gidx": gidx_all,
        "doff": doff_all,
        "xT": xT_all,
        "dis": dis_all,
        "consts": consts,
        "nodes_by_core": nodes_by_core,
    }


def _build(plan, stage=99):
    import concourse.bacc as bacc
    import concourse.bass as bass
    import concourse.tile as tile
    import concourse.mybir as mybir

    f32 = mybir.dt.float32
    bf16 = mybir.dt.bfloat16
    i16 = mybir.dt.int16
    AF = mybir.ActivationFunctionType
    ALU = mybir.AluOpType

    sched = plan["sched"]
    ngather_total = plan["ngather_total"]
    nmm_total = plan["nmm_total"]
    gcols = ngather_total * (BATCH // 16)
    max_ng = max(g["ngather"] for g in sched)

    nc = bacc.Bacc("TRN2", target_bir_lowering=False, debug=False,
                   num_devices=NCORES, num_swdge_queues=NQ)

    t_xT = nc.dram_tensor("xT", [128, S], f32, kind="ExternalInput")
    t_dis = nc.dram_tensor("dis", [128, NT], f32, kind="ExternalInput")
    t_W1 = nc.dram_tensor("W1", [FIN, HD], f32, kind="ExternalInput")
    t_W2b = nc.dram_tensor("W2b", [128, C], f32, kind="ExternalInput")
    t_b1 = nc.dram_tensor("b1b", [128, HD], f32, kind="ExternalInput")
    t_b2 = nc.dram_tensor("b2b", [128, C], f32, kind="ExternalInput")
    t_id2 = nc.dram_tensor("id2", [128, 64], f32, kind="ExternalInput")
    t_gi = nc.dram_tensor("gidx", [128, gcols], i16, kind="ExternalInput")
    t_do = nc.dram_tensor("doff", [128, nmm_total], f32, kind="ExternalInput")
    t_cn = nc.dram_tensor("consts", [128, 640], f32, kind="ExternalInput")
    t_y = nc.dram_tensor("y", [S, C], f32, kind="ExternalOutput")

    # tables are bf16 padded to 128 cols (gather elem granularity is 256B)
    g1_bounce = nc.dram_tensor("g1_bounce", [S, 128], bf16, kind="Internal")
    g2_bounce = nc.dram_tensor("g2_bounce", [S, 128], bf16, kind="Internal")
    g1_table = nc.dram_tensor("g1_table", [GROWS, 128], bf16, kind="Internal",
                              addr_space="Shared")
    g2_table = nc.dram_tensor("g2_table", [GROWS, 128], bf16, kind="Internal",
                              addr_space="Shared")

    with tile.TileContext(nc) as tc:
        with tc.tile_pool(name="sbc", bufs=1) as sbc, \
             tc.tile_pool(name="sbx", bufs=2) as sbx, \
             tc.tile_pool(name="sbg", bufs=6) as sbg, \
             tc.tile_pool(name="sbm", bufs=4) as sbm, \
             tc.tile_pool(name="sbi", bufs=2) as sbi, \
             tc.tile_pool(name="sbt", bufs=3) as sbt, \
             tc.tile_pool(name="psa", bufs=2, space="PSUM") as psa, \
             tc.tile_pool(name="pst", bufs=2, space="PSUM") as pst:

            W1t = sbc.tile([FIN, HD], f32)
            nc.sync.dma_start(out=W1t[:], in_=t_W1[:])
            W2bt = sbc.tile([128, C], f32)
            nc.sync.dma_start(out=W2bt[:], in_=t_W2b[:])
            b1t = sbc.tile([128, HD], f32)
            nc.sync.dma_start(out=b1t[:], in_=t_b1[:])
            b2t = sbc.tile([128, C], f32)
            nc.sync.dma_start(out=b2t[:], in_=t_b2[:])
            id2t = sbc.tile([128, 64], f32)
            nc.sync.dma_start(out=id2t[:], in_=t_id2[:])
            cons = sbc.tile([128, 640], f32)
            nc.sync.dma_start(out=cons[:], in_=t_cn[:])
            dist = sbc.tile([128, NT], f32)
            nc.sync.dma_start(out=dist[:], in_=t_dis[:])

            iota = cons[:, 0:128]
            zrow = cons[0:1, 512:640]      # zeros [1, 128]
            zrhs = cons[0:1, 128:640]      # zeros [1, 512]

            # ---------------- layer-1 prep: g1 = dis * (x @ W1) -------------
            NCHUNK = S // 1024             # 13
            for ch in range(NCHUNK):
                xc = sbx.tile([128, 1024], f32, tag="xc")
                nc.sync.dma_start(out=xc[:],
                                  in_=t_xT[:, ch * 1024:(ch + 1) * 1024])
                for j in range(8):
                    t = 8 * ch + j
                    pp = pst.tile([128, 128], f32, tag="pp")
                    nc.tensor.matmul(out=pp[:, 0:HD],
                                     lhsT=xc[:, j * 128:(j + 1) * 128],
                                     rhs=W1t[:], start=True, stop=True)
                    g1 = sbt.tile([128, 128], bf16, tag="g1")
                    nc.vector.tensor_scalar(
                        out=g1[:, 0:HD], in0=pp[:, 0:HD],
                        scalar1=dist[:, t:t + 1], scalar2=None, op0=ALU.mult)
                    nc.sync.dma_start(
                        out=g1_bounce[t * 128:(t + 1) * 128, :], in_=g1[:])

            if stage >= 2:
                nc.gpsimd.collective_compute(
                    "AllGather", mybir.AluOpType.bypass,
                    replica_groups=[list(range(NCORES))],
                    ins=[g1_bounce[:]], outs=[g1_table[:]])

            # ---------------- edge phase ----------------
            def edge_phase(table, bounce, layer):
                g_cursor = 0
                m_cursor = 0
                gq = 0
                for sbp in range(NSBP):
                    b0 = sbp * PAIRW
                    acc = psa.tile([128, HALF_COLS], f32, tag="acc")
                    # clear all 3 banks (both halves at once)
                    for seg in range(3):
                        nc.tensor.matmul(
                            out=acc[:, seg * 512:(seg + 1) * 512],
                            lhsT=zrow[:], rhs=zrhs[:],
                            start=True, stop=False, skip_group_check=True)
                    # find last mm per bank to set stop
                    grp_list = [g for g in sched if g["sbp"] == sbp]
                    last_of_seg = {}
                    mm_idx = 0
                    for grp in grp_list:
                        for b in grp["tiles"]:
                            lb = b - b0
                            seg = ((lb % HALF_BLKS) * 128) // 512
                            last_of_seg[seg] = mm_idx
                            mm_idx += 1
                    mm_idx = 0
                    for grp in grp_list:
                        r = grp["r"]
                        tiles = grp["tiles"]
                        if not tiles:
                            continue
                        r1 = min((r + 1) * RANGE_W, GROWS)
                        src_ap = table[r * RANGE_W:r1, :]
                        n_tiles = len(tiles)
                        ng = grp["ngather"]
                        gi = sbi.tile([128, max_ng * (BATCH // 16)], i16,
                                      tag="gi")
                        col0 = g_cursor * (BATCH // 16)
                        nc.sync.dma_start(
                            out=gi[:, 0:ng * (BATCH // 16)],
                            in_=t_gi[:, col0:col0 + ng * (BATCH // 16)])
                        for g in range(ng):
                            t0 = g * 8
                            t1 = min(t0 + 8, n_tiles)
                            nt8 = t1 - t0
                            nidx = nt8 * 128
                            gcol = g * (BATCH // 16)
                            buf = sbg.tile([128, 8, 128], bf16, tag="buf")
                            nc.gpsimd.dma_gather(
                                out_ap=buf[:, 0:nt8, :],
                                in_ap=src_ap,
                                idxs_ap=gi[:, gcol:gcol + nidx // 16],
                                num_idxs=nidx,
                                num_idxs_reg=nidx,
                                elem_size=128,
                                queue_num=gq % NQ,
                                single_packet=os.environ.get("KSP", "0") == "1",
                            )
                            gq += 1
                            # batched M build for this chunk's tiles
                            mm0 = m_cursor + t0
                            M = sbm.tile([128, 8, 128], bf16, tag="M")
                            do_sl = dof_sb[:, mm0 - m_base:mm0 - m_base + nt8]
                            nc.vector.tensor_tensor(
                                out=M[:, 0:nt8, :],
                                in0=iota.unsqueeze(1).broadcast_to(
                                    [128, nt8, 128]),
                                in1=do_sl.unsqueeze(2).broadcast_to(
                                    [128, nt8, 128]),
                                op=ALU.is_equal)
                            for j in range(nt8):
                                b = tiles[t0 + j]
                                lb = b - b0
                                half = lb // HALF_BLKS
                                col = (lb % HALF_BLKS) * 128
                                nc.tensor.matmul(
                                    out=acc[64 * half:64 * half + 64,
                                            col:col + 128],
                                    lhsT=buf[:, j, 0:HD],
                                    rhs=M[:, j, :],
                                    start=False,
                                    stop=(mm_idx == last_of_seg.get(
                                        (col // 512), -2)),
                                    skip_group_check=True,
                                    tile_position=(0, 64 * half))
                                mm_idx += 1
                        g_cursor += grp["ngather"]
                        m_cursor += n_tiles
                    # ---------------- tails for this sbp ----------------
                    nblk = min(PAIRW, NT - b0)
                    for lb in range(nblk):
                        b = b0 + lb
                        half = lb // HALF_BLKS
                        col = (lb % HALF_BLKS) * 128
                        hs, he = 64 * half, 64 * half + 64
                        if layer == 1:
                            pt = sbt.tile([128, 128], f32, tag="pt")
                            nc.scalar.copy(pt[hs:he, :],
                                           acc[hs:he, col:col + 128])
                            selfb = sbt.tile([128, 128], bf16, tag="selfb")
                            nc.scalar.dma_start(
                                out=selfb[:],
                                in_=bounce[b * 128:(b + 1) * 128, :])
                            tr = pst.tile([128, 128], f32, tag="pp")
                            nc.tensor.transpose(out=tr[:, 0:64],
                                                in_=pt[hs:he, :],
                                                identity=id2t[hs:he, :])
                            t0v = sbt.tile([128, HD], f32, tag="t0v")
                            nc.vector.tensor_tensor(
                                out=t0v[:], in0=tr[:, 0:64],
                                in1=selfb[:, 0:HD], op=ALU.add)
                            t1v = sbt.tile([128, HD], f32, tag="t1v")
                            nc.vector.scalar_tensor_tensor(
                                out=t1v[:], in0=t0v[:],
                                scalar=dist[:, b:b + 1],
                                in1=b1t[:], op0=ALU.mult, op1=ALU.add)
                            zt = sbt.tile([128, HD], f32, tag="zt")
                            nc.scalar.activation(zt[:], t1v[:], AF.Relu)
                            zs = sbt.tile([128, 2, HD], bf16, tag="zs")
                            # write z~ into both 64-col halves so the L2
                            # transposed self-load is valid on either
                            # partition half
                            nc.vector.tensor_scalar(
                                out=zs[:],
                                in0=zt[:].unsqueeze(1).broadcast_to(
                                    [128, 2, HD]),
                                scalar1=dist[:, b:b + 1], scalar2=None,
                                op0=ALU.mult)
                            nc.sync.dma_start(
                                out=g2_bounce[b * 128:(b + 1) * 128, :],
                                in_=zs[:])
                        else:
                            selfT = sbt.tile([128, 128], bf16, tag="selfT")
                            nc.sync.dma_start(
                                out=selfT[:],
                                in_=bounce[b * 128:(b + 1) * 128, :],
                                transpose=True)
                            pt = sbt.tile([128, 128], f32, tag="pt")
                            nc.vector.tensor_tensor(
                                out=pt[hs:he, :],
                                in0=acc[hs:he, col:col + 128],
                                in1=selfT[hs:he, :], op=ALU.add)
                            h2T = pst.tile([128, 128], f32, tag="pp")
                            nc.tensor.matmul(out=h2T[0:64, :],
                                             lhsT=W2bt[hs:he, :],
                                             rhs=pt[hs:he, :],
                                             start=True, stop=True,
                                             tile_position=(64 * half, 0))
                            h2Ts = sbt.tile([128, 128], f32, tag="h2Ts")
                            nc.scalar.copy(h2Ts[0:64, :], h2T[0:64, :])
                            h2 = pst.tile([128, 128], f32, tag="pp")
                            nc.tensor.transpose(out=h2[:, 0:64],
                                                in_=h2Ts[0:64, :],
                                                identity=id2t[0:64, :])
                            lg = sbt.tile([128, C], f32, tag="lg")
                            nc.vector.scalar_tensor_tensor(
                                out=lg[:], in0=h2[:, 0:64],
                                scalar=dist[:, b:b + 1],
                                in1=b2t[:], op0=ALU.mult, op1=ALU.add)
                            nmax = sbt.tile([128, 1], f32, tag="nmax")
                            nc.vector.tensor_reduce(
                                out=nmax[:], in_=lg[:],
                                axis=mybir.AxisListType.X,
                                op=ALU.max, negate=True)
                            ex = sbt.tile([128, C], f32, tag="ex")
                            sume = sbt.tile([128, 1], f32, tag="sume")
                            nc.scalar.activation(ex[:], lg[:], AF.Exp,
                                                 bias=nmax[:], scale=1.0,
                                                 accum_out=sume[:])
                            lse = sbt.tile([128, 1], f32, tag="lse")
                            nc.scalar.activation(lse[:], sume[:], AF.Ln)
                            cc = sbt.tile([128, 1], f32, tag="cc")
                            nc.vector.tensor_tensor(out=cc[:], in0=nmax[:],
                                                    in1=lse[:],
                                                    op=ALU.subtract)
                            yt = sbt.tile([128, C], f32, tag="yt")
                            nc.vector.tensor_scalar(
                                out=yt[:], in0=lg[:], scalar1=cc[:],
                                scalar2=None, op0=ALU.add)
                            nc.sync.dma_start(
                                out=t_y[b * 128:(b + 1) * 128, :], in_=yt[:])

            # load dstoff per layer once (small)
            dof_sb = sbc.tile([128, nmm_total], f32)
            nc.sync.dma_start(out=dof_sb[:], in_=t_do[:])
            m_base = 0

            if stage >= 3:
                edge_phase(g1_table, g1_bounce, 1)

            if stage >= 4:
                nc.gpsimd.collective_compute(
                    "AllGather", mybir.AluOpType.bypass,
                    replica_groups=[list(range(NCORES))],
                    ins=[g2_bounce[:]], outs=[g2_table[:]])

            if stage >= 5:
                edge_phase(g2_table, g2_bounce, 2)

    nc.compile()
    return nc


def _run(inputs, trace=False):
    import concourse.bass_utils as bass_utils

    x = np.asarray(inputs["x"], np.float32)
    W1 = np.asarray(inputs["W1"], np.float32)
    b1 = np.asarray(inputs["b1"], np.float32)
    W2 = np.asarray(inputs["W2"], np.float32)
    b2 = np.asarray(inputs["b2"], np.float32)

    plan = _plan(x, inputs["edge_index"])
    nc = _build(plan, stage=int(os.environ.get("KSTAGE", "99")))

    b1b = np.tile(b1[None, :], (128, 1)).astype(np.float32)
    b2b = np.tile(b2[None, :], (128, 1)).astype(np.float32)
    W2b = np.tile(W2, (2, 1)).astype(np.float32)
    id2 = np.tile(np.eye(64, dtype=np.float32), (2, 1))

    in_maps = []
    for c in range(NCORES):
        in_maps.append({
            "xT": plan["xT"][c],
            "dis": plan["dis"][c],
            "W1": W1, "W2b": W2b, "b1b": b1b, "b2b": b2b,
            "id2": id2,
            "gidx": plan["gidx"][c],
            "doff": plan["doff"][c],
            "consts": plan["consts"],
        })

    res = bass_utils.run_bass_kernel_spmd(
        nc, in_maps, core_ids=list(range(NCORES)), trace=trace)

    out = np.empty((N, C), np.float32)
    for c in range(NCORES):
        yc = np.asarray(res.results[c]["y"], np.float32)
        nodes_c = plan["nodes_by_core"][c]
        out[nodes_c] = yc[:len(nodes_c)]
    return out, res


def kernel(**inputs):
    out, _ = _run(inputs, trace=False)
    return out


# revision 26
# speedup vs baseline: 1.0371x; 1.0371x over previous
"""GCN 2-layer kernel for Trainium2 (8 NeuronCores, Bass/Tile), v2.

Strategy (vs v1 which used dma_scatter_add into DRAM accumulators):
  - Nodes sharded across 8 cores (degree-balanced snake), S=13312 slots/core
    = 104 blocks of 128 nodes.
  - Per layer: a node table g (g1 = dis*(x@W1), g2 = dis*z) is computed
    per-shard, AllGathered to a full [106496, 64] fp32 DRAM table.
    NOTE: the layer-2 W2 matmul is moved AFTER aggregation (linearity),
    so the layer-2 table is just dis*z.
  - Edge phase: edges (plus self-loops) are grouped by destination block
    (128 dsts) and source range (16K rows, int16 gather index limit), with
    token counts padded to a cross-core-uniform static schedule.
    dma_gather pulls source rows (256B each) into SBUF tiles of
    [128 tokens, 64 feats]; the TensorEngine segment-reduces each tile into
    a feature-major PSUM accumulator [64, 1536] (24-block superblock pairs
    on partition halves) via tile-built 0/1 matrices
    M[token, dst_off] = (dstoff[token] == iota), streamed as matmul rhs.
    PSUM banks are cleared by outer-product zero-matmuls (start=True),
    real matmuls accumulate with start=False.
  - Tails per 128-dst block: transpose to node-major via PE, then
    layer 1: z~ = dis*relu(dis*agg + b1) -> layer-2 table;
    layer 2: h2 = (agg @ W2), y = log_softmax(dis*h2 + b2).
"""
import os
import sys

sys.path.insert(0, "/opt/trn_rl_repo")

import numpy as np

N, FIN, HD, C = 100000, 128, 64, 64
E = 1600000
NCORES = 8
S = 13312                  # slots per core
NT = S // 128              # 104 dst blocks
GROWS = NCORES * S         # 106496 global table rows
RANGE_W = int(os.environ.get("KRW", "16384"))  # gather index range (int16)
NR = (GROWS + RANGE_W - 1) // RANGE_W
PAIRW = 24                 # dst blocks per superblock-pair (2 halves x 12)
NSBP = (NT + PAIRW - 1) // PAIRW          # 5 (24,24,24,24,8)
HALF_BLKS = PAIRW // 2     # 12
HALF_COLS = HALF_BLKS * 128  # 1536 psum cols per half (3 banks)
BATCH = 1024               # max tokens per gather
NQ = 4


def _plan(x, edge_index):
    """Host-side planning. Returns per-core arrays + a uniform schedule."""
    x = np.asarray(x, np.float32)
    ei = np.asarray(edge_index, np.int64)
    src, dst = ei[0], ei[1]

    deg = np.bincount(dst, minlength=N).astype(np.float64) + 1.0  # + self loop
    dis_f = (1.0 / np.sqrt(deg)).astype(np.float32)

    # degree-balanced snake sharding
    order = np.argsort(-deg, kind="stable")
    posn = np.arange(N)
    blk, rem = posn // NCORES, posn % NCORES
    corepat = np.where(blk % 2 == 0, rem, NCORES - 1 - rem).astype(np.int32)
    core_of = np.empty(N, np.int32)
    core_of[order] = corepat

    lrow_of = np.empty(N, np.int64)
    nodes_by_core = []
    for c in range(NCORES):
        nodes_c = order[core_of[order] == c]
        lrow_of[nodes_c] = np.arange(len(nodes_c))
        nodes_by_core.append(nodes_c)

    grow_of = core_of.astype(np.int64) * S + lrow_of

    # per-core token lists: real edges only (self loops get dedicated
    # "self" groups gathering from the local bounce tensor)
    tok_src = []   # global row of source
    tok_dst = []   # local row of dest
    dst_core = core_of[dst]
    for c in range(NCORES):
        m = dst_core == c
        tok_src.append(grow_of[src[m]])
        tok_dst.append(lrow_of[dst[m]])

    # counts per (core, block, range)
    cnt = np.zeros((NCORES, NT, NR), np.int64)
    for c in range(NCORES):
        b = tok_dst[c] >> 7
        r = tok_src[c] // RANGE_W
        np.add.at(cnt[c], (b, r), 1)

    # uniform padded tile counts per (block, range)
    ntok = cnt.max(axis=0)                       # [NT, NR]
    ntiles_br = -(-ntok // 128)                  # ceil to 128-token tiles

    # schedule: per sbp, per-range edge groups (self-loop contributions are
    # added in the tails from direct contiguous loads of the bounce tensor).
    # Tiles in block-major order; gathers chunk them by 8 (last partial).
    sched = []            # per (sbp, r): dict(tiles=[(b)], ngather)
    total_tiles = 0
    for sbp in range(NSBP):
        b0, b1 = sbp * PAIRW, min((sbp + 1) * PAIRW, NT)
        for r in range(NR):
            tiles = []
            for b in range(b0, b1):
                tiles += [b] * int(ntiles_br[b, r])
            ng = -(-len(tiles) // 8) if tiles else 0
            sched.append({
                "sbp": sbp, "r": r, "tiles": tiles, "ngather": ng,
            })
            total_tiles += len(tiles)

    ngather_total = sum(g["ngather"] for g in sched)
    nmm_total = total_tiles

    # build per-core gidx + dstoff arrays following the schedule
    def wrap16(v):
        n = len(v)
        a = np.asarray(v, np.int16).reshape(n // 16, 16).T.copy()
        return np.tile(a, (8, 1))

    gidx_all = np.zeros((NCORES, 128, ngather_total * (BATCH // 16)), np.int16)
    doff_all = np.full((NCORES, 128, max(nmm_total, 1)), -1.0, np.float32)

    for c in range(NCORES):
        b_of = tok_dst[c] >> 7
        r_of = tok_src[c] // RANGE_W
        # bucket tokens by (block, range)
        key = b_of * NR + r_of
        osort = np.argsort(key, kind="stable")
        ts_s, td_s = tok_src[c][osort], tok_dst[c][osort]
        key_s = key[osort]
        starts = np.searchsorted(key_s, np.arange(NT * NR))
        ends = np.searchsorted(key_s, np.arange(NT * NR) + 1)

        g_cursor = 0   # in gathers
        m_cursor = 0   # in mms (= tiles)
        for grp in sched:
            r = grp["r"]
            tiles = grp["tiles"]
            if not tiles:
                continue
            # assemble this group's token stream (tile-major, 128 per tile)
            n_tok = len(tiles) * 128
            idx_loc = np.zeros(n_tok, np.int64)   # gather idx within range
            dof = np.full(n_tok, -1.0, np.float32)
            # fill per block
            tpos = 0
            tiles_arr = np.asarray(tiles)
            for b in np.unique(tiles_arr):
                k = b * NR + r
                s0, s1 = starts[k], ends[k]
                cnt_b = s1 - s0
                nt_b = int(np.count_nonzero(tiles_arr == b))
                # tokens for this block go into its nt_b tiles
                off = tpos * 128
                idx_loc[off:off + cnt_b] = ts_s[s0:s1] - r * RANGE_W
                dof[off:off + cnt_b] = (
                    td_s[s0:s1] - b * 128).astype(np.float32)
                tpos += nt_b
            # scatter into gidx (per gather chunk) and dstoff (per tile)
            n_tiles = len(tiles)
            for g in range(grp["ngather"]):
                t0, t1 = g * 8, min((g + 1) * 8, n_tiles)
                nidx = (t1 - t0) * 128
                seg = idx_loc[t0 * 128: t0 * 128 + nidx]
                col0 = (g_cursor + g) * (BATCH // 16)
                w = wrap16(np.pad(seg, (0, BATCH - nidx)))
                gidx_all[c][:, col0:col0 + BATCH // 16] = w
            doff_all[c][:, m_cursor:m_cursor + n_tiles] = (
                dof.reshape(n_tiles, 128).T)
            g_cursor += grp["ngather"]
            m_cursor += n_tiles

    # per-core xT, dis with p = row%128, t = row//128
    xT_all = np.zeros((NCORES, 128, S), np.float32)
    dis_all = np.ones((NCORES, 128, NT), np.float32)
    for c in range(NCORES):
        nodes_c = nodes_by_core[c]
        n_real = len(nodes_c)
        lin = np.arange(n_real)
        xT_all[c][:, lin] = x[nodes_c].T
        dis_all[c][lin % 128, lin // 128] = dis_f[nodes_c]

    # consts tile: [128, 640]: cols 0:128 iota, 128:640 zeros
    consts = np.zeros((128, 640), np.float32)
    consts[:, 0:128] = np.arange(128, dtype=np.float32)[None, :]

    return {
        "sched": sched,
        "ngather_total": ngather_total,
        "nmm_total": nmm_total,
        "gidx": gidx_all,
        "doff": doff_all,
        "xT": xT_all,
        "dis": dis_all,
        "consts": consts,
        "nodes_by_core": nodes_by_core,
    }


def _build(plan, stage=99):
    import concourse.bacc as bacc
    import concourse.bass as bass
    import concourse.tile as tile
    import concourse.mybir as mybir

    f32 = mybir.dt.float32
    bf16 = mybir.dt.bfloat16
    i16 = mybir.dt.int16
    AF = mybir.ActivationFunctionType
    ALU = mybir.AluOpType

    sched = plan["sched"]
    ngather_total = plan["ngather_total"]
    nmm_total = plan["nmm_total"]
    gcols = ngather_total * (BATCH // 16)
    max_ng = max(g["ngather"] for g in sched)

    nc = bacc.Bacc("TRN2", target_bir_lowering=False, debug=False,
                   num_devices=NCORES, num_swdge_queues=NQ)

    t_xT = nc.dram_tensor("xT", [128, S], f32, kind="ExternalInput")
    t_dis = nc.dram_tensor("dis", [128, NT], f32, kind="ExternalInput")
    t_W1 = nc.dram_tensor("W1", [FIN, HD], f32, kind="ExternalInput")
    t_W2b = nc.dram_tensor("W2b", [128, C], f32, kind="ExternalInput")
    t_b1 = nc.dram_tensor("b1b", [128, HD], f32, kind="ExternalInput")
    t_b2 = nc.dram_tensor("b2b", [128, C], f32, kind="ExternalInput")
    t_id2 = nc.dram_tensor("id2", [128, 64], f32, kind="ExternalInput")
    t_gi = nc.dram_tensor("gidx", [128, gcols], i16, kind="ExternalInput")
    t_do = nc.dram_tensor("doff", [128, nmm_total], f32, kind="ExternalInput")
    t_cn = nc.dram_tensor("consts", [128, 640], f32, kind="ExternalInput")
    t_y = nc.dram_tensor("y", [S, C], f32, kind="ExternalOutput")

    # tables are bf16 padded to 128 cols (gather elem granularity is 256B)
    g1_bounce = nc.dram_tensor("g1_bounce", [S, 128], bf16, kind="Internal")
    g2_bounce = nc.dram_tensor("g2_bounce", [S, 128], bf16, kind="Internal")
    g1_table = nc.dram_tensor("g1_table", [GROWS, 128], bf16, kind="Internal",
                              addr_space="Shared")
    g2_table = nc.dram_tensor("g2_table", [GROWS, 128], bf16, kind="Internal",
                              addr_space="Shared")

    with tile.TileContext(nc) as tc:
        with tc.tile_pool(name="sbc", bufs=1) as sbc, \
             tc.tile_pool(name="sbx", bufs=2) as sbx, \
             tc.tile_pool(name="sbg", bufs=6) as sbg, \
             tc.tile_pool(name="sbm", bufs=4) as sbm, \
             tc.tile_pool(name="sbi", bufs=2) as sbi, \
             tc.tile_pool(name="sbt", bufs=3) as sbt, \
             tc.tile_pool(name="psa", bufs=2, space="PSUM") as psa, \
             tc.tile_pool(name="pst", bufs=2, space="PSUM") as pst:

            W1t = sbc.tile([FIN, HD], f32)
            nc.sync.dma_start(out=W1t[:], in_=t_W1[:])
            W2bt = sbc.tile([128, C], f32)
            nc.sync.dma_start(out=W2bt[:], in_=t_W2b[:])
            b1t = sbc.tile([128, HD], f32)
            nc.sync.dma_start(out=b1t[:], in_=t_b1[:])
            b2t = sbc.tile([128, C], f32)
            nc.sync.dma_start(out=b2t[:], in_=t_b2[:])
            id2t = sbc.tile([128, 64], f32)
            nc.sync.dma_start(out=id2t[:], in_=t_id2[:])
            cons = sbc.tile([128, 640], f32)
            nc.sync.dma_start(out=cons[:], in_=t_cn[:])
            dist = sbc.tile([128, NT], f32)
            nc.sync.dma_start(out=dist[:], in_=t_dis[:])

            iota = cons[:, 0:128]
            zrow = cons[0:1, 512:640]      # zeros [1, 128]
            zrhs = cons[0:1, 128:640]      # zeros [1, 512]

            # ---------------- layer-1 prep: g1 = dis * (x @ W1) -------------
            NCHUNK = S // 1024             # 13
            for ch in range(NCHUNK):
                xc = sbx.tile([128, 1024], f32, tag="xc")
                nc.sync.dma_start(out=xc[:],
                                  in_=t_xT[:, ch * 1024:(ch + 1) * 1024])
                for j in range(8):
                    t = 8 * ch + j
                    pp = pst.tile([128, 128], f32, tag="pp")
                    nc.tensor.matmul(out=pp[:, 0:HD],
                                     lhsT=xc[:, j * 128:(j + 1) * 128],
                                     rhs=W1t[:], start=True, stop=True)
                    g1 = sbt.tile([128, 128], bf16, tag="g1")
                    nc.vector.tensor_tensor(
                        out=g1[:, 0:HD], in0=pp[:, 0:HD],
                        in1=dist[:, t:t + 1].broadcast_to([128, HD]),
                        op=ALU.mult)
                    nc.sync.dma_start(
                        out=g1_bounce[t * 128:(t + 1) * 128, :], in_=g1[:])

            if stage >= 2:
                nc.gpsimd.collective_compute(
                    "AllGather", mybir.AluOpType.bypass,
                    replica_groups=[list(range(NCORES))],
                    ins=[g1_bounce[:]], outs=[g1_table[:]])

            # ---------------- edge phase ----------------
            def edge_phase(table, bounce, layer):
                g_cursor = 0
                m_cursor = 0
                gq = 0
                for sbp in range(NSBP):
                    b0 = sbp * PAIRW
                    acc = psa.tile([128, HALF_COLS], f32, tag="acc")
                    # clear all 3 banks (both halves at once)
                    for seg in range(3):
                        nc.tensor.matmul(
                            out=acc[:, seg * 512:(seg + 1) * 512],
                            lhsT=zrow[:], rhs=zrhs[:],
                            start=True, stop=False, skip_group_check=True)
                    # find last mm per bank to set stop
                    grp_list = [g for g in sched if g["sbp"] == sbp]
                    last_of_seg = {}
                    mm_idx = 0
                    for grp in grp_list:
                        for b in grp["tiles"]:
                            lb = b - b0
                            seg = ((lb % HALF_BLKS) * 128) // 512
                            last_of_seg[seg] = mm_idx
                            mm_idx += 1
                    mm_idx = 0
                    for grp in grp_list:
                        r = grp["r"]
                        tiles = grp["tiles"]
                        if not tiles:
                            continue
                        r1 = min((r + 1) * RANGE_W, GROWS)
                        src_ap = table[r * RANGE_W:r1, :]
                        n_tiles = len(tiles)
                        ng = grp["ngather"]
                        gi = sbi.tile([128, max_ng * (BATCH // 16)], i16,
                                      tag="gi")
                        col0 = g_cursor * (BATCH // 16)
                        nc.sync.dma_start(
                            out=gi[:, 0:ng * (BATCH // 16)],
                            in_=t_gi[:, col0:col0 + ng * (BATCH // 16)])
                        for g in range(ng):
                            t0 = g * 8
                            t1 = min(t0 + 8, n_tiles)
                            nt8 = t1 - t0
                            nidx = nt8 * 128
                            gcol = g * (BATCH // 16)
                            buf = sbg.tile([128, 8, 128], bf16, tag="buf")
                            nc.gpsimd.dma_gather(
                                out_ap=buf[:, 0:nt8, :],
                                in_ap=src_ap,
                                idxs_ap=gi[:, gcol:gcol + nidx // 16],
                                num_idxs=nidx,
                                num_idxs_reg=nidx,
                                elem_size=128,
                                queue_num=gq % NQ,
                                single_packet=os.environ.get("KSP", "0") == "1",
                            )
                            gq += 1
                            # batched M build for this chunk's tiles
                            mm0 = m_cursor + t0
                            M = sbm.tile([128, 8, 128], bf16, tag="M")
                            do_sl = dof_sb[:, mm0 - m_base:mm0 - m_base + nt8]
                            nc.vector.tensor_tensor(
                                out=M[:, 0:nt8, :],
                                in0=iota.unsqueeze(1).broadcast_to(
                                    [128, nt8, 128]),
                                in1=do_sl.unsqueeze(2).broadcast_to(
                                    [128, nt8, 128]),
                                op=ALU.is_equal)
                            for j in range(nt8):
                                b = tiles[t0 + j]
                                lb = b - b0
                                half = lb // HALF_BLKS
                                col = (lb % HALF_BLKS) * 128
                                nc.tensor.matmul(
                                    out=acc[64 * half:64 * half + 64,
                                            col:col + 128],
                                    lhsT=buf[:, j, 0:HD],
                                    rhs=M[:, j, :],
                                    start=False,
                                    stop=(mm_idx == last_of_seg.get(
                                        (col // 512), -2)),
                                    skip_group_check=True,
                                    tile_position=(0, 64 * half))
                                mm_idx += 1
                        g_cursor += grp["ngather"]
                        m_cursor += n_tiles
                    # ---------------- tails for this sbp ----------------
                    nblk = min(PAIRW, NT - b0)
                    for lb in range(nblk):
                        b = b0 + lb
                        half = lb // HALF_BLKS
                        col = (lb % HALF_BLKS) * 128
                        hs, he = 64 * half, 64 * half + 64
                        if layer == 1:
                            pt = sbt.tile([128, 128], f32, tag="pt")
                            nc.scalar.copy(pt[hs:he, :],
                                           acc[hs:he, col:col + 128])
                            selfb = sbt.tile([128, 128], bf16, tag="selfb")
                            nc.scalar.dma_start(
                                out=selfb[:],
                                in_=bounce[b * 128:(b + 1) * 128, :])
                            tr = pst.tile([128, 128], f32, tag="pp")
                            nc.tensor.transpose(out=tr[:, 0:64],
                                                in_=pt[hs:he, :],
                                                identity=id2t[hs:he, :])
                            t0v = sbt.tile([128, HD], f32, tag="t0v")
                            nc.vector.tensor_tensor(
                                out=t0v[:], in0=tr[:, 0:64],
                                in1=selfb[:, 0:HD], op=ALU.add)
                            t1m = sbt.tile([128, HD], f32, tag="t1m")
                            nc.vector.tensor_tensor(
                                out=t1m[:], in0=t0v[:],
                                in1=dist[:, b:b + 1].broadcast_to([128, HD]),
                                op=ALU.mult)
                            t1v = sbt.tile([128, HD], f32, tag="t1v")
                            nc.vector.tensor_tensor(
                                out=t1v[:], in0=t1m[:], in1=b1t[:],
                                op=ALU.add)
                            zt = sbt.tile([128, HD], f32, tag="zt")
                            nc.scalar.activation(zt[:], t1v[:], AF.Relu)
                            zs = sbt.tile([128, 2, HD], bf16, tag="zs")
                            # write z~ into both 64-col halves so the L2
                            # transposed self-load is valid on either
                            # partition half
                            nc.vector.tensor_tensor(
                                out=zs[:],
                                in0=zt[:].unsqueeze(1).broadcast_to(
                                    [128, 2, HD]),
                                in1=dist[:, b:b + 1].unsqueeze(2).broadcast_to(
                                    [128, 2, HD]),
                                op=ALU.mult)
                            nc.sync.dma_start(
                                out=g2_bounce[b * 128:(b + 1) * 128, :],
                                in_=zs[:])
                        else:
                            selfT = sbt.tile([128, 128], bf16, tag="selfT")
                            nc.sync.dma_start(
                                out=selfT[:],
                                in_=bounce[b * 128:(b + 1) * 128, :],
                                transpose=True)
                            pt = sbt.tile([128, 128], f32, tag="pt")
                            nc.vector.tensor_tensor(
                                out=pt[hs:he, :],
                                in0=acc[hs:he, col:col + 128],
                                in1=selfT[hs:he, :], op=ALU.add)
                            h2T = pst.tile([128, 128], f32, tag="pp")
                            nc.tensor.matmul(out=h2T[0:64, :],
                                             lhsT=W2bt[hs:he, :],
                                             rhs=pt[hs:he, :],
                                             start=True, stop=True,
                                             tile_position=(64 * half, 0))
                            h2Ts = sbt.tile([128, 128], f32, tag="h2Ts")
                            nc.scalar.copy(h2Ts[0:64, :], h2T[0:64, :])
                            h2 = pst.tile([128, 128], f32, tag="pp")
                            nc.tensor.transpose(out=h2[:, 0:64],
                                                in_=h2Ts[0:64, :],
                                                identity=id2t[0:64, :])
                            lgm = sbt.tile([128, C], f32, tag="lgm")
                            nc.vector.tensor_tensor(
                                out=lgm[:], in0=h2[:, 0:64],
                                in1=dist[:, b:b + 1].broadcast_to([128, C]),
                                op=ALU.mult)
                            lg = sbt.tile([128, C], f32, tag="lg")
                            nc.vector.tensor_tensor(
                                out=lg[:], in0=lgm[:], in1=b2t[:],
                                op=ALU.add)
                            nmax = sbt.tile([128, 1], f32, tag="nmax")
                            nc.vector.tensor_reduce(
                                out=nmax[:], in_=lg[:],
                                axis=mybir.AxisListType.X,
                                op=ALU.max, negate=True)
                            ex = sbt.tile([128, C], f32, tag="ex")
                            sume = sbt.tile([128, 1], f32, tag="sume")
                            nc.scalar.activation(ex[:], lg[:], AF.Exp,
                                                 bias=nmax[:], scale=1.0,
                                                 accum_out=sume[:])
                            lse = sbt.tile([128, 1], f32, tag="lse")
                            nc.scalar.activation(lse[:], sume[:], AF.Ln)
                            cc = sbt.tile([128, 1], f32, tag="cc")
                            nc.vector.tensor_tensor(out=cc[:], in0=nmax[:],
                                                    in1=lse[:],
                                                    op=ALU.subtract)
                            yt = sbt.tile([128, C], f32, tag="yt")
                            nc.vector.tensor_tensor(
                                out=yt[:], in0=lg[:],
                                in1=cc[:].broadcast_to([128, C]),
                                op=ALU.add)
                            nc.sync.dma_start(
                                out=t_y[b * 128:(b + 1) * 128, :], in_=yt[:])

            # load dstoff per layer once (small)
            dof_sb = sbc.tile([128, nmm_total], f32)
            nc.sync.dma_start(out=dof_sb[:], in_=t_do[:])
            m_base = 0

            if stage >= 3:
                edge_phase(g1_table, g1_bounce, 1)

            if stage >= 4:
                nc.gpsimd.collective_compute(
                    "AllGather", mybir.AluOpType.bypass,
                    replica_groups=[list(range(NCORES))],
                    ins=[g2_bounce[:]], outs=[g2_table[:]])

            if stage >= 5:
                edge_phase(g2_table, g2_bounce, 2)

    nc.compile()
    return nc


def _run(inputs, trace=False):
    import concourse.bass_utils as bass_utils

    x = np.asarray(inputs["x"], np.float32)
    W1 = np.asarray(inputs["W1"], np.float32)
    b1 = np.asarray(inputs["b1"], np.float32)
    W2 = np.asarray(inputs["W2"], np.float32)
    b2 = np.asarray(inputs["b2"], np.float32)

    plan = _plan(x, inputs["edge_index"])
    nc = _build(plan, stage=int(os.environ.get("KSTAGE", "99")))

    b1b = np.tile(b1[None, :], (128, 1)).astype(np.float32)
    b2b = np.tile(b2[None, :], (128, 1)).astype(np.float32)
    W2b = np.tile(W2, (2, 1)).astype(np.float32)
    id2 = np.tile(np.eye(64, dtype=np.float32), (2, 1))

    in_maps = []
    for c in range(NCORES):
        in_maps.append({
            "xT": plan["xT"][c],
            "dis": plan["dis"][c],
            "W1": W1, "W2b": W2b, "b1b": b1b, "b2b": b2b,
            "id2": id2,
            "gidx": plan["gidx"][c],
            "doff": plan["doff"][c],
            "consts": plan["consts"],
        })

    res = bass_utils.run_bass_kernel_spmd(
        nc, in_maps, core_ids=list(range(NCORES)), trace=trace)

    out = np.empty((N, C), np.float32)
    for c in range(NCORES):
        yc = np.asarray(res.results[c]["y"], np.float32)
        nodes_c = plan["nodes_by_core"][c]
        out[nodes_c] = yc[:len(nodes_c)]
    return out, res


def kernel(**inputs):
    out, _ = _run(inputs, trace=False)
    return out


# revision 33
# speedup vs baseline: 1.1104x; 1.0707x over previous
"""GCN 2-layer kernel for Trainium2 (8 NeuronCores, Bass/Tile), v2.

Strategy (vs v1 which used dma_scatter_add into DRAM accumulators):
  - Nodes sharded across 8 cores (degree-balanced snake), S=13312 slots/core
    = 104 blocks of 128 nodes.
  - Per layer: a node table g (g1 = dis*(x@W1), g2 = dis*z) is computed
    per-shard, AllGathered to a full [106496, 64] fp32 DRAM table.
    NOTE: the layer-2 W2 matmul is moved AFTER aggregation (linearity),
    so the layer-2 table is just dis*z.
  - Edge phase: edges (plus self-loops) are grouped by destination block
    (128 dsts) and source range (16K rows, int16 gather index limit), with
    token counts padded to a cross-core-uniform static schedule.
    dma_gather pulls source rows (256B each) into SBUF tiles of
    [128 tokens, 64 feats]; the TensorEngine segment-reduces each tile into
    a feature-major PSUM accumulator [64, 1536] (24-block superblock pairs
    on partition halves) via tile-built 0/1 matrices
    M[token, dst_off] = (dstoff[token] == iota), streamed as matmul rhs.
    PSUM banks are cleared by outer-product zero-matmuls (start=True),
    real matmuls accumulate with start=False.
  - Tails per 128-dst block: transpose to node-major via PE, then
    layer 1: z~ = dis*relu(dis*agg + b1) -> layer-2 table;
    layer 2: h2 = (agg @ W2), y = log_softmax(dis*h2 + b2).
"""
import os
import sys

sys.path.insert(0, "/opt/trn_rl_repo")

import numpy as np

N, FIN, HD, C = 100000, 128, 64, 64
E = 1600000
NCORES = 8
S = 13312                  # slots per core
NT = S // 128              # 104 dst blocks
GROWS = NCORES * S         # 106496 global table rows
RANGE_W = int(os.environ.get("KRW", "16384"))  # gather index range (int16)
NR = (GROWS + RANGE_W - 1) // RANGE_W
PAIRW = 24                 # dst blocks per superblock-pair (2 halves x 12)
NSBP = (NT + PAIRW - 1) // PAIRW          # 5 (24,24,24,24,8)
HALF_BLKS = PAIRW // 2     # 12
HALF_COLS = HALF_BLKS * 128  # 1536 psum cols per half (3 banks)
BATCH = 1024               # max tokens per gather
NQ = 4


def _plan(x, edge_index):
    """Host-side planning. Returns per-core arrays + a uniform schedule."""
    x = np.asarray(x, np.float32)
    ei = np.asarray(edge_index, np.int64)
    src, dst = ei[0], ei[1]

    deg = np.bincount(dst, minlength=N).astype(np.float64) + 1.0  # + self loop
    dis_f = (1.0 / np.sqrt(deg)).astype(np.float32)

    # degree-balanced snake sharding
    order = np.argsort(-deg, kind="stable")
    posn = np.arange(N)
    blk, rem = posn // NCORES, posn % NCORES
    corepat = np.where(blk % 2 == 0, rem, NCORES - 1 - rem).astype(np.int32)
    core_of = np.empty(N, np.int32)
    core_of[order] = corepat

    lrow_of = np.empty(N, np.int64)
    nodes_by_core = []
    for c in range(NCORES):
        nodes_c = order[core_of[order] == c]
        lrow_of[nodes_c] = np.arange(len(nodes_c))
        nodes_by_core.append(nodes_c)

    grow_of = core_of.astype(np.int64) * S + lrow_of

    # per-core token lists: real edges only (self loops get dedicated
    # "self" groups gathering from the local bounce tensor)
    tok_src = []   # global row of source
    tok_dst = []   # local row of dest
    dst_core = core_of[dst]
    for c in range(NCORES):
        m = dst_core == c
        tok_src.append(grow_of[src[m]])
        tok_dst.append(lrow_of[dst[m]])

    # counts per (core, block, range)
    cnt = np.zeros((NCORES, NT, NR), np.int64)
    for c in range(NCORES):
        b = tok_dst[c] >> 7
        r = tok_src[c] // RANGE_W
        np.add.at(cnt[c], (b, r), 1)

    # uniform padded tile counts per (block, range)
    ntok = cnt.max(axis=0)                       # [NT, NR]
    ntiles_br = -(-ntok // 128)                  # ceil to 128-token tiles

    # schedule: per sbp, per-range edge groups (self-loop contributions are
    # added in the tails from direct contiguous loads of the bounce tensor).
    # Tiles in block-major order; gathers chunk them by 8 (last partial).
    sched = []            # per (sbp, r): dict(tiles=[(b)], ngather)
    total_tiles = 0
    for sbp in range(NSBP):
        b0, b1 = sbp * PAIRW, min((sbp + 1) * PAIRW, NT)
        for r in range(NR):
            tiles = []
            for b in range(b0, b1):
                tiles += [b] * int(ntiles_br[b, r])
            ng = -(-len(tiles) // 8) if tiles else 0
            sched.append({
                "sbp": sbp, "r": r, "tiles": tiles, "ngather": ng,
            })
            total_tiles += len(tiles)

    ngather_total = sum(g["ngather"] for g in sched)
    nmm_total = total_tiles

    # build per-core gidx + dstoff arrays following the schedule
    def wrap16(v):
        n = len(v)
        a = np.asarray(v, np.int16).reshape(n // 16, 16).T.copy()
        return np.tile(a, (8, 1))

    gidx_all = np.zeros((NCORES, 128, ngather_total * (BATCH // 16)), np.int16)
    doff_all = np.full((NCORES, 128, max(nmm_total, 1)), -1.0, np.float32)

    for c in range(NCORES):
        b_of = tok_dst[c] >> 7
        r_of = tok_src[c] // RANGE_W
        # bucket tokens by (block, range)
        key = b_of * NR + r_of
        osort = np.argsort(key, kind="stable")
        ts_s, td_s = tok_src[c][osort], tok_dst[c][osort]
        key_s = key[osort]
        starts = np.searchsorted(key_s, np.arange(NT * NR))
        ends = np.searchsorted(key_s, np.arange(NT * NR) + 1)

        g_cursor = 0   # in gathers
        m_cursor = 0   # in mms (= tiles)
        for grp in sched:
            r = grp["r"]
            tiles = grp["tiles"]
            if not tiles:
                continue
            # assemble this group's token stream (tile-major, 128 per tile)
            n_tok = len(tiles) * 128
            idx_loc = np.zeros(n_tok, np.int64)   # gather idx within range
            dof = np.full(n_tok, -1.0, np.float32)
            # fill per block
            tpos = 0
            tiles_arr = np.asarray(tiles)
            for b in np.unique(tiles_arr):
                k = b * NR + r
                s0, s1 = starts[k], ends[k]
                cnt_b = s1 - s0
                nt_b = int(np.count_nonzero(tiles_arr == b))
                # tokens for this block go into its nt_b tiles
                off = tpos * 128
                idx_loc[off:off + cnt_b] = ts_s[s0:s1] - r * RANGE_W
                dof[off:off + cnt_b] = (
                    td_s[s0:s1] - b * 128).astype(np.float32)
                tpos += nt_b
            # scatter into gidx (per gather chunk) and dstoff (per tile)
            n_tiles = len(tiles)
            for g in range(grp["ngather"]):
                t0, t1 = g * 8, min((g + 1) * 8, n_tiles)
                nidx = (t1 - t0) * 128
                seg = idx_loc[t0 * 128: t0 * 128 + nidx]
                col0 = (g_cursor + g) * (BATCH // 16)
                w = wrap16(np.pad(seg, (0, BATCH - nidx)))
                gidx_all[c][:, col0:col0 + BATCH // 16] = w
            doff_all[c][:, m_cursor:m_cursor + n_tiles] = (
                dof.reshape(n_tiles, 128).T)
            g_cursor += grp["ngather"]
            m_cursor += n_tiles

    # per-core xT, dis with p = row%128, t = row//128
    xT_all = np.zeros((NCORES, 128, S), np.float32)
    dis_all = np.ones((NCORES, 128, NT), np.float32)
    for c in range(NCORES):
        nodes_c = nodes_by_core[c]
        n_real = len(nodes_c)
        lin = np.arange(n_real)
        xT_all[c][:, lin] = x[nodes_c].T
        dis_all[c][lin % 128, lin // 128] = dis_f[nodes_c]

    # consts tile: [128, 640]: cols 0:128 iota, 128:640 zeros
    consts = np.zeros((128, 640), np.float32)
    consts[:, 0:128] = np.arange(128, dtype=np.float32)[None, :]

    return {
        "sched": sched,
        "ngather_total": ngather_total,
        "nmm_total": nmm_total,
        "gidx": gidx_all,
        "doff": doff_all,
        "xT": xT_all,
        "dis": dis_all,
        "consts": consts,
        "nodes_by_core": nodes_by_core,
    }


def _build(plan, stage=99):
    import concourse.bacc as bacc
    import concourse.bass as bass
    import concourse.tile as tile
    import concourse.mybir as mybir

    f32 = mybir.dt.float32
    bf16 = mybir.dt.bfloat16
    i16 = mybir.dt.int16
    AF = mybir.ActivationFunctionType
    ALU = mybir.AluOpType

    sched = plan["sched"]
    ngather_total = plan["ngather_total"]
    nmm_total = plan["nmm_total"]
    gcols = ngather_total * (BATCH // 16)
    max_ng = max(g["ngather"] for g in sched)

    nc = bacc.Bacc("TRN2", target_bir_lowering=False, debug=False,
                   num_devices=NCORES, num_swdge_queues=NQ)

    t_xT = nc.dram_tensor("xT", [128, S], f32, kind="ExternalInput")
    t_dis = nc.dram_tensor("dis", [128, NT], f32, kind="ExternalInput")
    t_W1 = nc.dram_tensor("W1", [FIN, HD], f32, kind="ExternalInput")
    t_W2b = nc.dram_tensor("W2b", [128, C], f32, kind="ExternalInput")
    t_b1 = nc.dram_tensor("b1b", [128, HD], f32, kind="ExternalInput")
    t_b2 = nc.dram_tensor("b2b", [128, C], f32, kind="ExternalInput")
    t_id2 = nc.dram_tensor("id2", [128, 64], f32, kind="ExternalInput")
    t_idf = nc.dram_tensor("idf", [128, 128], mybir.dt.bfloat16,
                           kind="ExternalInput")
    t_gi = nc.dram_tensor("gidx", [128, gcols], i16, kind="ExternalInput")
    t_do = nc.dram_tensor("doff", [128, nmm_total], f32, kind="ExternalInput")
    t_cn = nc.dram_tensor("consts", [128, 640], f32, kind="ExternalInput")
    t_y = nc.dram_tensor("y", [S, C], f32, kind="ExternalOutput")

    # tables are bf16 padded to 128 cols (gather elem granularity is 256B)
    g1_bounce = nc.dram_tensor("g1_bounce", [S, 128], bf16, kind="Internal")
    g2_bounce = nc.dram_tensor("g2_bounce", [S, 128], bf16, kind="Internal")
    g1_table = nc.dram_tensor("g1_table", [GROWS, 128], bf16, kind="Internal",
                              addr_space="Shared")
    g2_table = nc.dram_tensor("g2_table", [GROWS, 128], bf16, kind="Internal",
                              addr_space="Shared")

    with tile.TileContext(nc) as tc:
        with tc.tile_pool(name="sbc", bufs=1) as sbc, \
             tc.tile_pool(name="sbx", bufs=2) as sbx, \
             tc.tile_pool(name="sbg", bufs=6) as sbg, \
             tc.tile_pool(name="sbm", bufs=4) as sbm, \
             tc.tile_pool(name="sbi", bufs=2) as sbi, \
             tc.tile_pool(name="sbt", bufs=3) as sbt, \
             tc.tile_pool(name="psa", bufs=2, space="PSUM") as psa, \
             tc.tile_pool(name="pst", bufs=2, space="PSUM") as pst:

            W1t = sbc.tile([FIN, HD], f32)
            nc.sync.dma_start(out=W1t[:], in_=t_W1[:])
            W2bt = sbc.tile([128, C], f32)
            nc.sync.dma_start(out=W2bt[:], in_=t_W2b[:])
            b1t = sbc.tile([128, HD], f32)
            nc.sync.dma_start(out=b1t[:], in_=t_b1[:])
            b2t = sbc.tile([128, C], f32)
            nc.sync.dma_start(out=b2t[:], in_=t_b2[:])
            id2t = sbc.tile([128, 64], f32)
            nc.sync.dma_start(out=id2t[:], in_=t_id2[:])
            idft = sbc.tile([128, 128], bf16)
            nc.sync.dma_start(out=idft[:], in_=t_idf[:])
            cons = sbc.tile([128, 640], f32)
            nc.sync.dma_start(out=cons[:], in_=t_cn[:])
            dist = sbc.tile([128, NT], f32)
            nc.sync.dma_start(out=dist[:], in_=t_dis[:])

            iota = cons[:, 0:128]
            zrow = cons[0:1, 512:640]      # zeros [1, 128]
            zrhs = cons[0:1, 128:640]      # zeros [1, 512]

            # ---------------- layer-1 prep: g1 = dis * (x @ W1) -------------
            NCHUNK = S // 1024             # 13
            for ch in range(NCHUNK):
                xc = sbx.tile([128, 1024], f32, tag="xc")
                nc.sync.dma_start(out=xc[:],
                                  in_=t_xT[:, ch * 1024:(ch + 1) * 1024])
                for j in range(8):
                    t = 8 * ch + j
                    pp = pst.tile([128, 128], f32, tag="pp")
                    nc.tensor.matmul(out=pp[:, 0:HD],
                                     lhsT=xc[:, j * 128:(j + 1) * 128],
                                     rhs=W1t[:], start=True, stop=True)
                    g1 = sbt.tile([128, 128], bf16, tag="g1")
                    nc.vector.tensor_tensor(
                        out=g1[:, 0:HD], in0=pp[:, 0:HD],
                        in1=dist[:, t:t + 1].broadcast_to([128, HD]),
                        op=ALU.mult)
                    nc.sync.dma_start(
                        out=g1_bounce[t * 128:(t + 1) * 128, :], in_=g1[:])

            if stage >= 2:
                nc.gpsimd.collective_compute(
                    "AllGather", mybir.AluOpType.bypass,
                    replica_groups=[list(range(NCORES))],
                    ins=[g1_bounce[:]], outs=[g1_table[:]])

            # ---------------- edge phase ----------------
            def edge_phase(table, bounce, layer):
                g_cursor = 0
                m_cursor = 0
                gq = 0
                for sbp in range(NSBP):
                    b0 = sbp * PAIRW
                    acc = psa.tile([128, HALF_COLS], f32, tag="acc")
                    # clear all 3 banks (both halves at once)
                    for seg in range(3):
                        nc.tensor.matmul(
                            out=acc[:, seg * 512:(seg + 1) * 512],
                            lhsT=zrow[:], rhs=zrhs[:],
                            start=True, stop=False, skip_group_check=True)
                    # self-loop contribution: acc[block] += own_g.T via
                    # accumulate-matmul with identity rhs
                    for lb in range(min(PAIRW, NT - b0)):
                        b = b0 + lb
                        half = lb // HALF_BLKS
                        col = (lb % HALF_BLKS) * 128
                        selfb = sbt.tile([128, 128], bf16, tag="selfb")
                        nc.scalar.dma_start(
                            out=selfb[:],
                            in_=bounce[b * 128:(b + 1) * 128, :])
                        nc.tensor.matmul(
                            out=acc[64 * half:64 * half + 64, col:col + 128],
                            lhsT=selfb[:, 0:HD], rhs=idft[:],
                            start=False, stop=False, skip_group_check=True,
                            tile_position=(0, 64 * half))
                    # find last mm per bank to set stop
                    grp_list = [g for g in sched if g["sbp"] == sbp]
                    last_of_seg = {}
                    mm_idx = 0
                    for grp in grp_list:
                        for b in grp["tiles"]:
                            lb = b - b0
                            seg = ((lb % HALF_BLKS) * 128) // 512
                            last_of_seg[seg] = mm_idx
                            mm_idx += 1
                    mm_idx = 0
                    for grp in grp_list:
                        r = grp["r"]
                        tiles = grp["tiles"]
                        if not tiles:
                            continue
                        r1 = min((r + 1) * RANGE_W, GROWS)
                        src_ap = table[r * RANGE_W:r1, :]
                        n_tiles = len(tiles)
                        ng = grp["ngather"]
                        gi = sbi.tile([128, max_ng * (BATCH // 16)], i16,
                                      tag="gi")
                        col0 = g_cursor * (BATCH // 16)
                        nc.sync.dma_start(
                            out=gi[:, 0:ng * (BATCH // 16)],
                            in_=t_gi[:, col0:col0 + ng * (BATCH // 16)])
                        for g in range(ng):
                            t0 = g * 8
                            t1 = min(t0 + 8, n_tiles)
                            nt8 = t1 - t0
                            nidx = nt8 * 128
                            gcol = g * (BATCH // 16)
                            buf = sbg.tile([128, 8, 128], bf16, tag="buf")
                            nc.gpsimd.dma_gather(
                                out_ap=buf[:, 0:nt8, :],
                                in_ap=src_ap,
                                idxs_ap=gi[:, gcol:gcol + nidx // 16],
                                num_idxs=nidx,
                                num_idxs_reg=nidx,
                                elem_size=128,
                                queue_num=gq % NQ,
                                single_packet=os.environ.get("KSP", "0") == "1",
                            )
                            gq += 1
                            # batched M build for this chunk's tiles
                            mm0 = m_cursor + t0
                            M = sbm.tile([128, 8, 128], bf16, tag="M")
                            do_sl = dof_sb[:, mm0 - m_base:mm0 - m_base + nt8]
                            nc.vector.tensor_tensor(
                                out=M[:, 0:nt8, :],
                                in0=iota.unsqueeze(1).broadcast_to(
                                    [128, nt8, 128]),
                                in1=do_sl.unsqueeze(2).broadcast_to(
                                    [128, nt8, 128]),
                                op=ALU.is_equal)
                            for j in range(nt8):
                                b = tiles[t0 + j]
                                lb = b - b0
                                half = lb // HALF_BLKS
                                col = (lb % HALF_BLKS) * 128
                                nc.tensor.matmul(
                                    out=acc[64 * half:64 * half + 64,
                                            col:col + 128],
                                    lhsT=buf[:, j, 0:HD],
                                    rhs=M[:, j, :],
                                    start=False,
                                    stop=(mm_idx == last_of_seg.get(
                                        (col // 512), -2)),
                                    skip_group_check=True,
                                    tile_position=(0, 64 * half))
                                mm_idx += 1
                        g_cursor += grp["ngather"]
                        m_cursor += n_tiles
                    # ---------------- tails for this sbp ----------------
                    nblk = min(PAIRW, NT - b0)
                    for lb in range(nblk):
                        b = b0 + lb
                        half = lb // HALF_BLKS
                        col = (lb % HALF_BLKS) * 128
                        hs, he = 64 * half, 64 * half + 64
                        if layer == 1:
                            pt = sbt.tile([128, 128], f32, tag="pt")
                            nc.scalar.copy(pt[hs:he, :],
                                           acc[hs:he, col:col + 128])
                            tr = pst.tile([128, 128], f32, tag="pp")
                            nc.tensor.transpose(out=tr[:, 0:64],
                                                in_=pt[hs:he, :],
                                                identity=id2t[hs:he, :])
                            t1m = sbt.tile([128, HD], f32, tag="t1m")
                            nc.vector.tensor_tensor(
                                out=t1m[:], in0=tr[:, 0:64],
                                in1=dist[:, b:b + 1].broadcast_to([128, HD]),
                                op=ALU.mult)
                            t1v = sbt.tile([128, HD], f32, tag="t1v")
                            nc.vector.tensor_tensor(
                                out=t1v[:], in0=t1m[:], in1=b1t[:],
                                op=ALU.add)
                            zt = sbt.tile([128, HD], f32, tag="zt")
                            nc.scalar.activation(zt[:], t1v[:], AF.Relu)
                            zs = sbt.tile([128, 2, HD], bf16, tag="zs")
                            # write z~ into both 64-col halves so the L2
                            # transposed self-load is valid on either
                            # partition half
                            nc.vector.tensor_tensor(
                                out=zs[:],
                                in0=zt[:].unsqueeze(1).broadcast_to(
                                    [128, 2, HD]),
                                in1=dist[:, b:b + 1].unsqueeze(2).broadcast_to(
                                    [128, 2, HD]),
                                op=ALU.mult)
                            nc.sync.dma_start(
                                out=g2_bounce[b * 128:(b + 1) * 128, :],
                                in_=zs[:])
                        else:
                            pt = sbt.tile([128, 128], f32, tag="pt")
                            nc.scalar.copy(pt[hs:he, :],
                                           acc[hs:he, col:col + 128])
                            h2T = pst.tile([128, 128], f32, tag="pp")
                            nc.tensor.matmul(out=h2T[0:64, :],
                                             lhsT=W2bt[hs:he, :],
                                             rhs=pt[hs:he, :],
                                             start=True, stop=True,
                                             tile_position=(64 * half, 0))
                            h2Ts = sbt.tile([128, 128], f32, tag="h2Ts")
                            nc.scalar.copy(h2Ts[0:64, :], h2T[0:64, :])
                            h2 = pst.tile([128, 128], f32, tag="pp")
                            nc.tensor.transpose(out=h2[:, 0:64],
                                                in_=h2Ts[0:64, :],
                                                identity=id2t[0:64, :])
                            lgm = sbt.tile([128, C], f32, tag="lgm")
                            nc.vector.tensor_tensor(
                                out=lgm[:], in0=h2[:, 0:64],
                                in1=dist[:, b:b + 1].broadcast_to([128, C]),
                                op=ALU.mult)
                            lg = sbt.tile([128, C], f32, tag="lg")
                            nc.vector.tensor_tensor(
                                out=lg[:], in0=lgm[:], in1=b2t[:],
                                op=ALU.add)
                            nmax = sbt.tile([128, 1], f32, tag="nmax")
                            nc.vector.tensor_reduce(
                                out=nmax[:], in_=lg[:],
                                axis=mybir.AxisListType.X,
                                op=ALU.max, negate=True)
                            ex = sbt.tile([128, C], f32, tag="ex")
                            sume = sbt.tile([128, 1], f32, tag="sume")
                            nc.scalar.activation(ex[:], lg[:], AF.Exp,
                                                 bias=nmax[:], scale=1.0,
                                                 accum_out=sume[:])
                            lse = sbt.tile([128, 1], f32, tag="lse")
                            nc.scalar.activation(lse[:], sume[:], AF.Ln)
                            cc = sbt.tile([128, 1], f32, tag="cc")
                            nc.vector.tensor_tensor(out=cc[:], in0=nmax[:],
                                                    in1=lse[:],
                                                    op=ALU.subtract)
                            yt = sbt.tile([128, C], f32, tag="yt")
                            nc.vector.tensor_tensor(
                                out=yt[:], in0=lg[:],
                                in1=cc[:].broadcast_to([128, C]),
                                op=ALU.add)
                            nc.sync.dma_start(
                                out=t_y[b * 128:(b + 1) * 128, :], in_=yt[:])

            # load dstoff per layer once (small)
            dof_sb = sbc.tile([128, nmm_total], f32)
            nc.sync.dma_start(out=dof_sb[:], in_=t_do[:])
            m_base = 0

            if stage >= 3:
                edge_phase(g1_table, g1_bounce, 1)

            if stage >= 4:
                nc.gpsimd.collective_compute(
                    "AllGather", mybir.AluOpType.bypass,
                    replica_groups=[list(range(NCORES))],
                    ins=[g2_bounce[:]], outs=[g2_table[:]])

            if stage >= 5:
                edge_phase(g2_table, g2_bounce, 2)

    nc.compile()
    return nc


def _run(inputs, trace=False):
    import concourse.bass_utils as bass_utils

    x = np.asarray(inputs["x"], np.float32)
    W1 = np.asarray(inputs["W1"], np.float32)
    b1 = np.asarray(inputs["b1"], np.float32)
    W2 = np.asarray(inputs["W2"], np.float32)
    b2 = np.asarray(inputs["b2"], np.float32)

    plan = _plan(x, inputs["edge_index"])
    nc = _build(plan, stage=int(os.environ.get("KSTAGE", "99")))

    import ml_dtypes
    b1b = np.tile(b1[None, :], (128, 1)).astype(np.float32)
    b2b = np.tile(b2[None, :], (128, 1)).astype(np.float32)
    W2b = np.tile(W2, (2, 1)).astype(np.float32)
    id2 = np.tile(np.eye(64, dtype=np.float32), (2, 1))
    idf = np.eye(128, dtype=ml_dtypes.bfloat16)

    in_maps = []
    for c in range(NCORES):
        in_maps.append({
            "xT": plan["xT"][c],
            "dis": plan["dis"][c],
            "W1": W1, "W2b": W2b, "b1b": b1b, "b2b": b2b,
            "id2": id2, "idf": idf,
            "gidx": plan["gidx"][c],
            "doff": plan["doff"][c],
            "consts": plan["consts"],
        })

    res = bass_utils.run_bass_kernel_spmd(
        nc, in_maps, core_ids=list(range(NCORES)), trace=trace)

    out = np.empty((N, C), np.float32)
    for c in range(NCORES):
        yc = np.asarray(res.results[c]["y"], np.float32)
        nodes_c = plan["nodes_by_core"][c]
        out[nodes_c] = yc[:len(nodes_c)]
    return out, res


def kernel(**inputs):
    out, _ = _run(inputs, trace=False)
    return out
